# revision 29
# baseline (speedup 1.0000x reference)
"""Trainium2 Bass kernel for nn_MultiHeadedAttention_4269197492266.

Dual-branch multi-head attention where the "local" key path is a multi-scale
conv (k=3,5) + batchnorm + projection.  Host-side algebra folds the whole
local path into a single 5-tap convolution:

    kl = bn(concat(conv3(key), conv5(key))) @ wkl.T + bkl
       = conv5tap(key, W5c) + bkl_eff

with W5c[o,i,d] = A5[o,i,d] + A3[o,i,d-1] (A* = wkl-slice @ (bn_scale * conv_w*)).
Work shards over (batch, head-group): core c handles batch c//2, heads
4*(c%2) .. 4*(c%2)+4; each core emits the partial output projection of its 4
heads and the host adds the two partials per batch plus the folded bias.

On-chip layout is feature-major ([d, L]); scores are computed transposed
([Lk, Lq]) so AV needs no transposes; a ones-column appended to V makes the
softmax denominator fall out of the same PSUM accumulation.

The projected q/kl/kg are stored as power-of-2-scaled fp8e4 in a
[dk-half(32) x 2-plane] layout so every score matmul runs in DoubleRow perf
mode (2 k-planes per instruction at half the cycles-per-row); the scale is
undone by the exp activation's scale argument.

The emission is a software-pipelined slot stream: one slot = one score
DoubleRow pair + its exp.  The AV matmuls of phase k-1 ride in slots 0-13 of
phase k (5 then 4 per slot, norms batched at slot 13); projection/outproj
work is chopped into ~1-2-matmul units pulled by a debt budget with
DMA-arrival-aware ready gating, so the ACT engine (the bottleneck: one
1024-wide exp per slot, ~267us total) never starves.  Softmax normalization
uses a partition-shifted reciprocal straight into partition 0 (no DMA hop)
+ gpsimd broadcast, batched recips->broadcasts->mults per phase.  Inputs
arrive via a single ordered DMA queue in criticality order (the cost model
serializes all DMA copies), column-chunked so the first exp fires ~9us in;
the final phase runs c-major with its own AV trailing by two slots so only
a short normalize+outproj tail follows the last exp.
"""

import math
from contextlib import ExitStack

import ml_dtypes
import numpy as np

import concourse.tile as tile
from concourse import bacc, mybir
from concourse import bass_utils

F32 = mybir.dt.float32
BF16 = mybir.dt.bfloat16
FP8 = mybir.dt.float8e4
BF16_NP = ml_dtypes.bfloat16
FP8_NP = ml_dtypes.float8_e4m3

B, L, D = 4, 2048, 512
H, DK = 8, 64
N_CORES = 8
HG = 4              # heads per core
DO = HG * DK        # 256 output dims per core
BN_EPS = 1e-5
NJ = D // 128       # 4 input-dim tiles
NLT = L // 128      # 16 L tiles of 128
NLQ = L // 512      # 4 lq blocks of 512

ET_BUFS = 27

# fp8 store scales (powers of 2); undone inside exp via its scale arg
SQ = 128.0          # q values ~N(0, 0.057^2)
SKL = 32.0          # kl values ~N(0, 0.56^2)
SKG = 16.0          # kg values ~N(0, 0.45^2)
EXP_SCALE = {0: 1.0 / (SQ * SKL),   # local branch
             1: 1.0 / (SQ * SKG)}   # global branch

# phase order: (pair, branch, lq-half); global branches first per half so the
# cheap kg projections start the exp stream and klT has time to fill in
PHASES = [(0, 1, 0), (1, 1, 0), (0, 0, 0), (1, 0, 0),
          (0, 1, 1), (1, 1, 1), (0, 0, 1), (1, 0, 1)]

_cache = {}


def _build_program(repeat=1, stages='all'):
    """Build + compile the per-core Bass program (same program on all cores)."""
    nc = bacc.Bacc("TRN2", target_bir_lowering=False, debug=False,
                   num_devices=N_CORES)

    dt_in = {}
    dt_in["xq"] = nc.dram_tensor("xq", [D, L], BF16, kind="ExternalInput").ap()
    dt_in["xk"] = nc.dram_tensor("xk", [D, L], FP8, kind="ExternalInput").ap()
    dt_in["xv"] = nc.dram_tensor("xv", [D, L], BF16, kind="ExternalInput").ap()
    dt_in["wq"] = nc.dram_tensor("wq", [D, DO], BF16, kind="ExternalInput").ap()
    dt_in["wk5"] = nc.dram_tensor("wk5", [5, D, DO], FP8, kind="ExternalInput").ap()
    dt_in["wkg"] = nc.dram_tensor("wkg", [D, DO], FP8, kind="ExternalInput").ap()
    dt_in["wv"] = nc.dram_tensor("wv", [D, DO], BF16, kind="ExternalInput").ap()
    dt_in["wo2"] = nc.dram_tensor("wo2", [128, 2, D], BF16, kind="ExternalInput").ap()
    dt_in["bkl"] = nc.dram_tensor("bkl", [DO], F32, kind="ExternalInput").ap()
    out_ap = nc.dram_tensor("out", [L, D], F32, kind="ExternalOutput").ap()

    with tile.TileContext(nc) as tc, ExitStack() as ctx:
        et = ctx.enter_context(tc.tile_pool(name="et", bufs=ET_BUFS))
        proj = ctx.enter_context(tc.tile_pool(name="projsb", bufs=1))
        norm = ctx.enter_context(tc.tile_pool(name="norm", bufs=8))
        ostage = ctx.enter_context(tc.tile_pool(name="ostage", bufs=3))
        sp = ctx.enter_context(tc.tile_pool(name="sp", bufs=2, space="PSUM"))
        work = ctx.enter_context(tc.tile_pool(name="work", bufs=4, space="PSUM"))

        # ---- persistent SBUF tensors (single-buffer pools) ----
        wq_sb = proj.tile([128, NJ, DO], BF16, tag="wq")
        wk5_sb = proj.tile([128, 5, NJ, DO], FP8, tag="wk5")
        wkg_sb = proj.tile([128, NJ, DO], FP8, tag="wkg")
        wv_sb = proj.tile([128, NJ, DO], BF16, tag="wv")
        wo2_sb = proj.tile([128, 2, D], BF16, tag="wo2")
        bkl_sb = proj.tile([128, 2], F32, tag="bkl")
        # fp8 q/kl/kg: [head-in-pair(2)x32 partitions, m-half, dk-plane, L]
        q8_sb = proj.tile([64, 2, 2, L], FP8, tag="q8")
        kl8_sb = proj.tile([64, 2, 2, L], FP8, tag="kl8")
        kg8_sb = proj.tile([64, 2, 2, L], FP8, tag="kg8")
        v_sb = proj.tile([128, NLT, HG, DK + 1], BF16, tag="v")
        xT_sb = [proj.tile([128, 2, L], BF16, tag=f"xT{br}", name=f"xT{br}")
                 for br in range(2)]

        warm = proj.tile([1, 16], F32, tag="warm")
        nc.vector.memset(warm[:], 0.0)
        nc.scalar.activation(warm[:], warm[:], mybir.ActivationFunctionType.Exp)

        def emit_body():
            LKP = L + 4  # padded length
            # ---- input DMAs split across queues for a fast warmup ----
            xqt = proj.tile([128, NJ, LKP], BF16, tag="xqt")
            kx8 = proj.tile([128, NJ, LKP], FP8, tag="kx8")
            xvt = proj.tile([128, NJ, LKP], BF16, tag="xvt")
            nc.vector.memset(kx8[:, :, 0:2], 0.0)
            nc.vector.memset(kx8[:, :, 2 + L:], 0.0)
            # PE p-state pre-warm: junk matmuls while input DMAs stream
            jk = proj.tile([128, 512], BF16, tag="jk")
            nc.vector.memset(jk[:], 0.0)
            nc.vector.memset(v_sb[:, :, :, DK:DK + 1], 1.0)
            for _w in range(6):
                wps = work.tile([128, 512], F32, tag="wk", name="warm_ps")
                nc.tensor.matmul(wps[:], jk[:, 0:128], jk[:], start=True, stop=True)
            # single ordered queue: criticality order, column-chunked inputs
            xq_r = dt_in["xq"].rearrange("(j p) l -> p j l", p=128)
            xk_r = dt_in["xk"].rearrange("(j p) l -> p j l", p=128)
            xv_r = dt_in["xv"].rearrange("(j p) l -> p j l", p=128)
            nc.sync.dma_start(wq_sb[:], dt_in["wq"].rearrange("(j p) o -> p j o", p=128))
            nc.sync.dma_start(xqt[:, 0:2, 0:512], xq_r[:, 0:2, 0:512])
            nc.sync.dma_start(xqt[:, 2:4, 0:512], xq_r[:, 2:4, 0:512])
            nc.sync.dma_start(wkg_sb[:], dt_in["wkg"].rearrange("(j p) o -> p j o", p=128))
            nc.sync.dma_start(kx8[:, 0:2, 2:2 + 512], xk_r[:, 0:2, 0:512])
            nc.sync.dma_start(kx8[:, 2:4, 2:2 + 512], xk_r[:, 2:4, 0:512])
            for cb in range(1, 4):
                nc.sync.dma_start(kx8[:, :, 2 + cb * 512:2 + cb * 512 + 512],
                                  xk_r[:, :, cb * 512:cb * 512 + 512])
            def xq_dma(cb):
                nc.sync.dma_start(xqt[:, :, cb * 512:cb * 512 + 512],
                                  xq_r[:, :, cb * 512:cb * 512 + 512])

            def xv_dma(cb):
                nc.sync.dma_start(xvt[:, :, cb * 512:cb * 512 + 512],
                                  xv_r[:, :, cb * 512:cb * 512 + 512])
            xq_dma(1)
            nc.sync.dma_start(bkl_sb[:], dt_in["bkl"].rearrange("(m p) -> p m", p=128))
            nc.sync.dma_start(wv_sb[:], dt_in["wv"].rearrange("(j p) o -> p j o", p=128))
            xv_dma(0)
            xv_dma(1)
            xv_dma(2)
            xq_dma(2)
            xv_dma(3)
            xq_dma(3)
            nc.sync.dma_start(wk5_sb[:],
                              dt_in["wk5"].rearrange("t (j p) o -> p t j o", p=128))
            nc.sync.dma_start(wo2_sb[:], dt_in["wo2"])

            # ---- small emission helpers ----
            def store8(dst8, ps, m, qb, scale, bias=None, eng=None):
                eng = eng or nc.vector
                cs = slice(qb * 512, qb * 512 + 512)
                if bias is None:
                    eng.tensor_scalar_mul(dst8[0:64, m, 0, cs], ps[0:64, :], scale)
                    eng.tensor_scalar_mul(dst8[0:64, m, 1, cs], ps[64:128, :], scale)
                else:
                    nc.vector.tensor_scalar(dst8[0:64, m, 0, cs], ps[0:64, :],
                                            bias[0:64, m:m + 1], scale,
                                            mybir.AluOpType.add, mybir.AluOpType.mult)
                    nc.vector.tensor_scalar(dst8[0:64, m, 1, cs], ps[64:128, :],
                                            bias[64:128, m:m + 1], scale,
                                            mybir.AluOpType.add, mybir.AluOpType.mult)

            # ---- filler machinery: chunks of (cost, closure) units ----
            chunks = []
            readys = []
            marks = {}
            pos = [0, 0]
            debt = [-2500.0]
            gslot = [0]

            def _emit_one():
                ci, ui = pos
                cost, fn = chunks[ci][ui]
                fn()
                debt[0] += cost
                if ui + 1 == len(chunks[ci]):
                    pos[0], pos[1] = ci + 1, 0
                else:
                    pos[1] = ui + 1

            def pull():
                n = 0
                cap = 1 if gslot[0] < 32 else 4
                while pos[0] < len(chunks) and n < cap:
                    if pos[1] == 0 and gslot[0] < readys[pos[0]]:
                        break
                    cost = chunks[pos[0]][pos[1]][0]
                    if gslot[0] >= 32 and debt[0] + cost > 5000:
                        break
                    _emit_one()
                    n += 1

            def drain(name):
                tgt = marks[name]
                while pos[0] < tgt:
                    _emit_one()

            def finish_chunk():
                while pos[1] != 0:
                    _emit_one()

            # ---- unit builders ----
            def proj_chunk_units(dst8, w_sb, m, qb, scale, off=0, bias=None):
                # bf16 path (q projection): 4 plain matmuls over j tiles
                box = []

                def u1():
                    ps = work.tile([128, 512], F32, tag="wk")
                    box.append(ps)
                    for j in range(2):
                        nc.tensor.matmul(ps[:], w_sb[:, j, m * 128:(m + 1) * 128],
                                         xqt[:, j, off + qb * 512:off + qb * 512 + 512],
                                         start=(j == 0), stop=False)

                def u2():
                    ps = box[0]
                    for j in range(2, 4):
                        nc.tensor.matmul(ps[:], w_sb[:, j, m * 128:(m + 1) * 128],
                                         xqt[:, j, off + qb * 512:off + qb * 512 + 512],
                                         start=False, stop=(j == 3))
                    store8(dst8, ps, m, qb, scale, bias=bias)
                return [(1024, u1), (1024, u2)]

            def kg8_units(m, qb, eng=None):
                # fp8 DoubleRow: 2 instructions contract all 4 j tiles
                def u():
                    ps = work.tile([128, 512], F32, tag="wk")
                    for jj in range(2):
                        nc.tensor.matmul(
                            ps[:], wkg_sb[:, 2 * jj:2 * jj + 2, m * 128:(m + 1) * 128],
                            kx8[:, 2 * jj:2 * jj + 2, 2 + qb * 512:2 + qb * 512 + 512],
                            start=(jj == 0), stop=(jj == 1),
                            perf_mode=mybir.MatmulPerfMode.DoubleRow)
                    store8(kg8_sb, ps, m, qb, SKG / 32.0, eng=eng)
                return [(768, u)]

            def klT_units(m, qb):
                # fp8 DoubleRow 5-tap conv: 10 instructions (2 j-planes each)
                box = []
                units = []
                for i in range(3):
                    def u(i=i):
                        if i == 0:
                            box.append(work.tile([128, 512], F32, tag="wk", name="klT_ps"))
                        ps = box[0]
                        ts = ((0, 1), (2, 3), (4,))[i]
                        for t in ts:
                            for jj in range(2):
                                sh = qb * 512 + t
                                nc.tensor.matmul(
                                    ps[:], wk5_sb[:, t, 2 * jj:2 * jj + 2, m * 128:(m + 1) * 128],
                                    kx8[:, 2 * jj:2 * jj + 2, sh:sh + 512],
                                    start=(t == 0 and jj == 0), stop=(t == 4 and jj == 1),
                                    perf_mode=mybir.MatmulPerfMode.DoubleRow)
                        if i == 2:
                            store8(kl8_sb, ps, m, qb, SKL / 32.0, bias=bkl_sb)
                    units.append(((1024, 1024, 512)[i], u))
                return units

            def v_unit(lt):
                def u():
                    ps = work.tile([128, 512], F32, tag="wk")
                    for j in range(NJ):
                        nc.tensor.matmul(ps[:, :DO], xvt[:, j, lt * 128:lt * 128 + 128],
                                         wv_sb[:, j, :],
                                         start=(j == 0), stop=(j == NJ - 1))
                    nc.vector.tensor_copy(
                        v_sb[:, lt, :, 0:DK],
                        ps[:, :DO].rearrange("p (h d) -> p h d", h=HG))
                return [(1024, u)]

            def outproj_units(lt):
                box = []

                def u1():
                    po = work.tile([128, 512], F32, tag="wk", name="po")
                    box.append(po)
                    for k, (br, c2) in enumerate(((0, 0), (0, 1))):
                        nc.tensor.matmul(
                            po[:], xT_sb[br][:, c2, lt * 128:lt * 128 + 128],
                            wo2_sb[:, c2, :], start=(k == 0), stop=False)

                def u2():
                    po = box[0]
                    for k, (br, c2) in enumerate(((1, 0), (1, 1))):
                        nc.tensor.matmul(
                            po[:], xT_sb[br][:, c2, lt * 128:lt * 128 + 128],
                            wo2_sb[:, c2, :], start=False, stop=(k == 1))
                    ot = ostage.tile([128, D], F32, tag="ot")
                    nc.vector.tensor_copy(ot[:], po[:])
                    nc.sync.dma_start(out_ap[lt * 128:lt * 128 + 128, :], ot[:])
                return [(1024, u1), (1024, u2)]

            def outproj_tile(lt):
                for cost, fn in outproj_units(lt):
                    fn()

            # ---- scores / AV / norm emission ----
            def slot(p, br, c, lk, eT, avjobs, do_pull=True):
                ps = sp.tile([128, 1024], F32, tag="sp")
                kT8 = kl8_sb if br == 0 else kg8_sb
                for hh in range(2):
                    pb = 32 * hh
                    nc.tensor.matmul(
                        ps[:, hh * 512:hh * 512 + 512],
                        kT8[pb:pb + 32, p, :, lk * 128:lk * 128 + 128],
                        q8_sb[pb:pb + 32, p, :, c * 512:c * 512 + 512],
                        start=True, stop=True,
                        perf_mode=mybir.MatmulPerfMode.DoubleRow)
                e_t = et.tile([128, 1024], BF16, tag="et")
                nc.scalar.activation(e_t[:], ps[:],
                                     mybir.ActivationFunctionType.Exp,
                                     scale=EXP_SCALE[br])
                eT[(lk, c)] = e_t
                gslot[0] += 1
                debt[0] += 512
                for job in avjobs:
                    job()
                debt[0] -= 2490
                if do_pull:
                    pull()

            def av_phase_alloc(p, qh, eT):
                avs = [[work.tile([DK + 1, 512], F32, tag="wk",
                                  name=f"av{hh}_{i}") for i in range(2)]
                       for hh in range(2)]
                flat = []
                for lk in range(NLT):
                    for hh in range(2):
                        for i in range(2):
                            def f(lk=lk, hh=hh, i=i):
                                nc.tensor.matmul(
                                    avs[hh][i][:], v_sb[:, lk, 2 * p + hh, :],
                                    eT[(lk, 2 * qh + i)][:, hh * 512:hh * 512 + 512],
                                    start=(lk == 0), stop=(lk == NLT - 1))
                                debt[0] += 512
                            flat.append(f)
                return avs, flat

            def norm_batch(items):
                # items: list of (av, br, hh, p, c); batched: DVE recips,
                # Pool broadcasts, DVE mults — no serial per-chain hops
                rds, bcs = [], []
                for av, br, hh, p, c in items:
                    rd0 = norm.tile([1, 512], F32, tag="rd0")
                    nc.vector.reciprocal(rd0[0:1, :], av[DK:DK + 1, :])
                    rds.append(rd0)
                for (av, br, hh, p, c), rd0 in zip(items, rds):
                    bc = norm.tile([DK, 512], F32, tag="bc")
                    nc.gpsimd.partition_broadcast(bc[:], rd0[0:1, :])
                    bcs.append(bc)
                for (av, br, hh, p, c), bc in zip(items, bcs):
                    nc.vector.tensor_tensor(
                        xT_sb[br][64 * hh:64 * hh + 64, p, c * 512:c * 512 + 512],
                        av[0:DK, :], bc[:], mybir.AluOpType.mult)

            def av_norms(avs, p, br, qh):
                norm_batch([(avs[hh][i], br, hh, p, 2 * qh + i)
                            for hh in range(2) for i in range(2)])

            # ---- build the filler queue (ready = DMA-arrival slot) ----
            def add(units, ready=0, mark=None):
                chunks.append(units)
                readys.append(ready)
                if mark is not None:
                    marks[mark] = len(chunks)

            for qb in (1, 2, 3):
                add(kg8_units(0, qb))
            for qb in range(NLQ):
                add(kg8_units(1, qb))
            add(proj_chunk_units(q8_sb, wq_sb, 1, 0, SQ))
            add(proj_chunk_units(q8_sb, wq_sb, 0, 1, SQ), ready=2)
            add(proj_chunk_units(q8_sb, wq_sb, 1, 1, SQ), ready=2)
            for lt in range(4):
                add(v_unit(lt), ready=5)
            for lt in range(4, 8):
                add(v_unit(lt), ready=6)
            for lt in range(8, 12):
                add(v_unit(lt), ready=8)
            add(proj_chunk_units(q8_sb, wq_sb, 0, 2, SQ), ready=9)
            add(proj_chunk_units(q8_sb, wq_sb, 1, 2, SQ), ready=9)
            for lt in range(12, 16):
                add(v_unit(lt), ready=11)
            add(proj_chunk_units(q8_sb, wq_sb, 0, 3, SQ), ready=12)
            add(proj_chunk_units(q8_sb, wq_sb, 1, 3, SQ), ready=12,
                mark='P1')
            for qb in range(NLQ):
                add(klT_units(0, qb), ready=16,
                    mark='P2' if qb == 3 else None)
            for qb in range(NLQ):
                add(klT_units(1, qb), ready=16,
                    mark='P3' if qb == 3 else None)
            marks['P4'] = len(chunks)
            marks['P5'] = len(chunks)
            for lt in range(8):
                add(outproj_units(lt), ready=150,
                    mark='P7' if lt == 7 else None)

            # ---- prologue: the chunks P0 slot 0 needs, emitted directly ----
            for cost, fn in proj_chunk_units(q8_sb, wq_sb, 0, 0, SQ):
                fn()
            for cost, fn in kg8_units(0, 0):
                fn()

            # ---- main slot stream ----
            eTs = []
            for k in range(7):
                p, br, qh = PHASES[k]
                finish_chunk()
                mk_name = f'P{k}'
                if mk_name in marks:
                    drain(mk_name)
                avs = avflat = None
                if k >= 1:
                    pp, pbr, pqh = PHASES[k - 1]
                    avs, avflat = av_phase_alloc(pp, pqh, eTs[k - 1])
                eT = {}
                a0 = 0
                for s in range(32):
                    if avs is None:
                        lk, ci = s % 16, s // 16   # c-major warmup order
                    else:
                        lk, ci = s // 2, s % 2
                    jobs = []
                    if avs is not None and s < 14:
                        n = 5 if s < 8 else 4
                        jobs = avflat[a0:a0 + n]
                        a0 += n
                    slot(p, br, 2 * qh + ci, lk, eT, jobs,
                         do_pull=(avs is None or s >= 16))
                    if avs is not None and s == 13:
                        av_norms(avs, pp, pbr, pqh)
                eTs.append(eT)

            # ---- P7 = (1, 0, 1): c-major stream with trailing self-AV ----
            p, br, qh = PHASES[7]
            finish_chunk()
            drain('P7')
            pp, pbr, pqh = PHASES[6]
            avs6, av6flat = av_phase_alloc(pp, pqh, eTs[6])
            eT = {}
            a0 = 0
            for s in range(16):
                jobs = []
                if s < 14:
                    n = 5 if s < 8 else 4
                    jobs = av6flat[a0:a0 + n]
                    a0 += n
                slot(p, br, 2, s, eT, jobs, do_pull=False)
                if s == 13:
                    av_norms(avs6, pp, pbr, pqh)
            av7c2 = [work.tile([DK + 1, 512], F32, tag="wk", name=f"a2_{hh}")
                     for hh in range(2)]
            av7c3 = [work.tile([DK + 1, 512], F32, tag="wk", name=f"a3_{hh}")
                     for hh in range(2)]

            def av_c(avs2, c, lk):
                def f():
                    for hh in range(2):
                        nc.tensor.matmul(
                            avs2[hh][:], v_sb[:, lk, 2 * p + hh, :],
                            eT[(lk, c)][:, hh * 512:hh * 512 + 512],
                            start=(lk == 0), stop=(lk == NLT - 1))
                    debt[0] += 1024
                return f

            for s in range(16):
                jobs = [av_c(av7c2, 2, s)]
                if s >= 4:
                    jobs.append(av_c(av7c3, 3, s - 4))
                slot(p, br, 3, s, eT, jobs)
            norm_batch([(av7c2[hh], br, hh, p, 2) for hh in range(2)])
            for lt in range(8, 12):
                outproj_tile(lt)
            for lk in range(12, 16):
                av_c(av7c3, 3, lk)()
            # narrow normalization: per-128-col pieces so each outproj tile
            # unlocks as soon as its columns are normalized
            rds3, bcs3 = [], []
            for hh in range(2):
                rd0 = norm.tile([1, 512], F32, tag="rd0", name=f"rdt{hh}")
                nc.vector.reciprocal(rd0[0:1, :], av7c3[hh][DK:DK + 1, :])
                rds3.append(rd0)
            for hh in range(2):
                bc = norm.tile([DK, 512], F32, tag="bc", name=f"bct{hh}")
                nc.gpsimd.partition_broadcast(bc[:], rds3[hh][0:1, :])
                bcs3.append(bc)
            for lt in range(12, 16):
                cs = slice((lt - 12) * 128, (lt - 12) * 128 + 128)
                for hh in range(2):
                    nc.vector.tensor_tensor(
                        xT_sb[br][64 * hh:64 * hh + 64, p,
                                  3 * 512 + (lt - 12) * 128:3 * 512 + (lt - 12) * 128 + 128],
                        av7c3[hh][0:DK, cs], bcs3[hh][:, cs], mybir.AluOpType.mult)
                outproj_tile(lt)

        for _rep in range(repeat):
            emit_body()

    nc.compile()
    return nc


# permutation of each 128-col m-half so the projection psum partitions land
# as [A dk0-31 | B dk0-31 | A dk32-63 | B dk32-63] (A=even head, B=odd head)
_PERM128 = np.concatenate([np.arange(0, 32), np.arange(64, 96),
                           np.arange(32, 64), np.arange(96, 128)])
_PERM256 = np.concatenate([_PERM128, 128 + _PERM128])


def _host_prep(inputs):
    """Fold conv+bn+biases; build the 8 per-core input maps."""
    f32 = np.float32
    q = np.ascontiguousarray(inputs["query"], dtype=f32)
    k = np.ascontiguousarray(inputs["key"], dtype=f32)
    v = np.ascontiguousarray(inputs["value"], dtype=f32)
    w3 = np.asarray(inputs["conv_w3"], f32)
    w5 = np.asarray(inputs["conv_w5"], f32)
    b3 = np.asarray(inputs["conv_b3"], f32)
    b5 = np.asarray(inputs["conv_b5"], f32)
    gam = np.asarray(inputs["bn_gamma"], f32)
    bet = np.asarray(inputs["bn_beta"], f32)
    mu = np.asarray(inputs["bn_mean"], f32)
    var = np.asarray(inputs["bn_var"], f32)
    wq = np.asarray(inputs["wq"], f32)
    bq = np.asarray(inputs["bq"], f32)
    wkl = np.asarray(inputs["wkl"], f32)
    bkl = np.asarray(inputs["bkl"], f32)
    wkg = np.asarray(inputs["wkg"], f32)
    bkg = np.asarray(inputs["bkg"], f32)
    wv = np.asarray(inputs["wv"], f32)
    bv = np.asarray(inputs["bv"], f32)
    wo = np.asarray(inputs["wo"], f32)
    bo = np.asarray(inputs["bo"], f32)

    # biases that would change the math in ways we don't model on-chip
    assert not np.any(bq) and not np.any(bkg), "nonzero q/kg bias unsupported"

    s_bn = gam / np.sqrt(var + BN_EPS)                       # [1024]
    shift = np.concatenate([b3, b5]) * s_bn + (bet - mu * s_bn)
    wkl_s = wkl * s_bn[None, :]                              # [512, 1024]
    A3 = np.einsum("oc,cit->oit", wkl_s[:, :D], w3)          # [512, 512, 3]
    A5 = np.einsum("oc,cit->oit", wkl_s[:, D:], w5)          # [512, 512, 5]
    W5c = A5.copy()
    W5c[:, :, 1:4] += A3
    bkl_eff = wkl @ shift + bkl                              # [512]
    wq_eff = wq / math.sqrt(DK)
    bo_eff = bo + wo @ (2.0 * bv)

    bf = BF16_NP
    in_maps = []
    for c in range(N_CORES):
        b = c // 2
        hg = c % 2
        sel = slice(hg * DO, hg * DO + DO)
        wo_l = wo.T[sel, :]                                  # [256, 512]
        wo2 = wo_l.reshape(2, 2, DK, D).transpose(1, 2, 0, 3).reshape(128, 2, D)
        in_maps.append({
            "xq": np.ascontiguousarray(q[b].T).astype(bf),
            "xk": np.ascontiguousarray(k[b].T).astype(FP8_NP),
            "xv": np.ascontiguousarray(v[b].T).astype(bf),
            "wq": np.ascontiguousarray(wq_eff.T[:, sel][:, _PERM256]).astype(bf),
            "wk5": np.ascontiguousarray(
                W5c.transpose(2, 1, 0)[:, :, sel][:, :, _PERM256] * 32.0).astype(FP8_NP),
            "wkg": np.ascontiguousarray(wkg.T[:, sel][:, _PERM256] * 32.0).astype(FP8_NP),
            "wv": np.ascontiguousarray(wv.T[:, sel]).astype(bf),
            "wo2": np.ascontiguousarray(wo2).astype(bf),
            "bkl": np.ascontiguousarray(bkl_eff[sel][_PERM256] * 32.0).astype(f32),
        })
    return in_maps, bo_eff


def kernel(**inputs) -> np.ndarray:
    if "nc" not in _cache:
        _cache["nc"] = _build_program()
    nc = _cache["nc"]
    in_maps, bo_eff = _host_prep(inputs)
    res = bass_utils.run_bass_kernel_spmd(
        nc, in_maps, core_ids=list(range(N_CORES)))
    out = np.zeros((B, L, D), np.float32)
    for c in range(N_CORES):
        out[c // 2] += res.results[c]["out"]
    out += bo_eff[None, None, :]
    return out
